# revision 45
# baseline (speedup 1.0000x reference)
"""Single-head attention (B=4, N=4096, E=1024, H=64) on 8 TRN2 NeuronCores.

Sharding: core c = (batch b = c//2, query-half h = c%2). Each core computes the
full K/V projections for its batch and attention for its 2048 query rows.
Attention is permutation-invariant over keys, so each core receives its batch's
x pre-transposed ([E, N], embedding on partitions) with its OWN query half in
columns 0:2048 - the program is identical across cores (pure SPMD), only the
data differs.

The kernel is ScalarE-bound: softmax needs exp over 8.4M elements per core and
exp only exists on ScalarE - a hard ~66-73us floor at GRP=2 activation sizing.
Everything else is organized to hide under that:
  - x and wT ship from the host already in bf16 (same rounding the device cast
    produced before): the x stream halves to 8 MiB and the on-chip
    fp32->bf16 casts disappear entirely.
  - ScalarE does NOTHING but exp: the q cross-duplication DMAs are replaced by
    computing q into both PSUM partition halves on the PE (two sequential
    double-chains - a start=True matmul clears the has_written bits of its
    output partitions across the whole bank, so a partition range may host
    only one in-flight accumulation chain), and softmax denominators are
    replicated across partitions with a K=1 PE matmul (bf16) instead of a
    DRAM broadcast bounce.
  - per block, k/q/v projections are column-packed 2x on the PE: each matmul
    is split by 128-column sub-chunk parity, even sub-chunks through column
    group 0 (PSUM partitions 0:64), odd through group 1 (64:128), so each key
    chunk drains straight into the partition half its S-matmul row-group reads.
    vT is PE-transposed into V-natural [128, 65] tiles with a fused ones
    column (softmax denominators ride the PV accumulation).
  - S^T groups pair an even chunk (rows 0:64) with its odd neighbor (64:128):
    the two K=64 matmuls run concurrently in the 128x128 array. One
    exp((q.k)/8) pass per group on ScalarE -> bf16 P; PV matmuls are deferred
    through a FIFO and flushed into PE spare cycles (late-stream blocks, then
    two per drain group), so the stream phase stays ScalarE-paced.
  - query blocks 0/1 ride chunk availability during the x stream; qb2/qb3
    drain through the same pools afterwards, their O accumulators reusing the
    then-idle projection and transpose PSUM banks.
  - junk matmuls + one junk exp at t0 open the HAM clock gate and load the
    exp spline tables off the critical path; a few more junk matmuls anchored
    on block 0/1's kT drains keep the activity window alive through the
    early stream (Tile hoists dependency-free warmups to t0 otherwise).
Host assembles out[b, half] = outT.T.
"""

import tempfile

import ml_dtypes
import numpy as np

import concourse.bass as bass
import concourse.tile as tile
from concourse import bacc, mybir
from concourse.bass_utils import run_bass_kernel_spmd
from concourse.masks import make_identity

B, N, E, H = 4, 4096, 1024, 64
NCORES = 8
NQ = N // 2  # query rows per core
QB = 512  # query block (free dim of attention matmuls)
NKC = N // 128  # 32 key chunks of 128
ECH = E // 128  # 8 embedding chunks of 128
NB = N // QB  # 8 projection column blocks
QBLKS = NQ // QB  # 4 query blocks per core
GRP = 2  # key chunks per S/exp group (PSUM banks per S tile)
NGROUPS = NKC // GRP  # 16 S/exp groups per query block

F32 = mybir.dt.float32
BF16 = mybir.dt.bfloat16

SCALE = 1.0 / np.sqrt(H)


def build_kernel():
    nc = bacc.Bacc("TRN2", target_bir_lowering=False, debug=False, num_devices=NCORES)

    xT_d = nc.dram_tensor("xT", [E, N], BF16, kind="ExternalInput")
    wT_d = nc.dram_tensor("wT", [E, 3 * H], BF16, kind="ExternalInput")
    outT_d = nc.dram_tensor("outT", [H, NQ], F32, kind="ExternalOutput")

    xT = xT_d.ap().rearrange("(c p) n -> p c n", p=128)  # [128, ECH, N]
    wT = wT_d.ap().rearrange("(c p) h -> p c h", p=128)  # [128, ECH, 192]
    outT = outT_d.ap()

    with tile.TileContext(nc) as tc:
        with (
            tc.tile_pool(name="singles", bufs=1) as singles,
            tc.tile_pool(name="xpool", bufs=4) as xpool,
            tc.tile_pool(name="qkv", bufs=1) as qkv,
            tc.tile_pool(name="vstage", bufs=2) as vstage,
            tc.tile_pool(name="ppool", bufs=26) as ppool,
            tc.tile_pool(name="npool", bufs=2) as npool,
            tc.tile_pool(name="kvq_ps", bufs=1, space="PSUM") as kvq_pool,
            tc.tile_pool(name="tr_ps", bufs=1, space="PSUM") as tr_pool,
            tc.tile_pool(name="s_ps", bufs=2, space="PSUM") as s_pool,
            tc.tile_pool(name="o_ps", bufs=2, space="PSUM") as o_pool,
        ):
            # x block 0 streams in quarter-pieces with wT slotted after the
            # first quarter: the kv chain consumes (wT, quarter0) first, so
            # this order lets it start ~1us earlier than wT-first
            # wT rides the (otherwise idle pre-exp) scalar HWDGE queue so it
            # lands in parallel with x block 0's quarters on the sync queue —
            # the kv chain consumes both
            wT_sb = singles.tile([128, ECH, 3 * H], BF16)
            nc.scalar.dma_start(out=wT_sb[:], in_=wT)
            x_t0 = xpool.tile([128, ECH, QB], BF16, name="x_t")
            for piece in range(4):
                nc.sync.dma_start(
                    out=x_t0[:, 2 * piece : 2 * piece + 2, :],
                    in_=xT[:, 2 * piece : 2 * piece + 2, 0:QB],
                )
            # identity (both halves) for PE transposes of vT
            ident = singles.tile([128, H], BF16)
            make_identity(nc, ident[0:H, :])
            nc.scalar.dma_start(out=ident[H : 2 * H, :], in_=ident[0:H, :])

            # persistent activations: k/q in both partition halves by layout,
            # V-natural tiles with a fused ones column
            kT_sb = qkv.tile([128, N], BF16)
            qT_sb = qkv.tile([128, NQ], BF16)
            # chunk stride padded to 80 (transposing DMA needs aligned outputs)
            # chunk stride padded to 128 with zeros past the ones column: the
            # PV stationary becomes a full 128-column weight, which satisfies
            # the compiler's FWL (fast weight load) condition — the junk rows
            # 65:128 of the O banks accumulate 0*P and are never read
            v_all = qkv.tile([128, NKC, 128], BF16)
            nc.vector.memset(v_all[:, :, H : H + 1], 1.0)
            nc.vector.memset(v_all[:, :, H + 1 :], 0.0)

            ones_h = singles.tile([1, H], BF16)
            nc.vector.memset(ones_h[:], 1.0)

            # PE warmup from t~0: junk matmuls on a memset tile (not gated
            # by the wT DMA) so the HAM clock-gate opens before real work;
            # one junk exp loads the ACT spline tables off the critical path
            junk = singles.tile([128, 256], BF16)
            nc.vector.memset(junk[:], 0.5)
            warm_act = singles.tile([1, 128], BF16)
            nc.scalar.activation(
                warm_act[:], junk[0:1, 0:128],
                mybir.ActivationFunctionType.Exp, scale=SCALE,
            )
            # warm into the transpose bank: the kvq bank must stay free so the
            # first projection chain isn't queued behind junk. 14 matmuls
            # bridge the PE to the first DMA-paced kv chain; 8 more are
            # emitted after it (filling the kT-drain window) so the HAM
            # activity window sees ~4us of sustained work and opens the clock
            # gate before the q/S chains
            warm_ps = tr_pool.tile([128, 192], F32, name="warm_ps", tag="tr")

            def warm(n):
                for _ in range(n):
                    nc.tensor.matmul(
                        warm_ps[0:H, :], junk[:, 0:H], junk[:, 64:256],
                        start=True, stop=True, tile_position=(0, 0),
                    )

            warm(14)

            o_acc = [None] * QBLKS
            next_chunk = [0] * QBLKS
            # PV work is deferred through a FIFO: the stream phase emits only
            # S+exp (keeping ScalarE the pacer) and PVs flush into the PE's
            # spare cycles — opportunistically late in the stream, then two
            # per group in the drain. The newest entry is never flushed so
            # the PE FIFO can't block on the in-flight exp.
            pv_queue = []

            def flush_pv(n):
                avail = len(pv_queue) - 1
                for _ in range(min(n, max(0, avail))):
                    emit_pv(*pv_queue.pop(0))

            def flush_pv_all():
                while pv_queue:
                    emit_pv(*pv_queue.pop(0))

            def s_matmul(s_slice, c, qsl):
                # even chunks live in partitions 0:64, odd chunks in 64:128
                lo = c % 2 == 0
                r = slice(0, H) if lo else slice(H, 2 * H)
                nc.tensor.matmul(
                    s_slice,
                    kT_sb[r, c * 128 : (c + 1) * 128],
                    qT_sb[r, qsl],
                    start=True, stop=True,
                    tile_position=(0 if lo else H, 0),
                )

            def group_chunks(i):
                # adjacent chunks pair lo/hi row-groups within one x block
                return [2 * i, 2 * i + 1]

            def emit_pv(qb, chunks, p_t):
                for j, c in enumerate(chunks):
                    nc.tensor.matmul(
                        o_acc[qb][:],
                        v_all[:, c, :],
                        p_t[:, j * QB : (j + 1) * QB],
                        start=(c == 0), stop=(c == NKC - 1),
                    )

            def emit_group(qb, i):
                if o_acc[qb] is None:
                    o_acc[qb] = o_pool.tile(
                        [128, QB], F32, name=f"o_qb{qb}", tag="o_acc"
                    )
                qsl = slice(qb * QB, (qb + 1) * QB)
                chunks = group_chunks(i)
                s_t = s_pool.tile([128, GRP * QB], F32, name="s_t")
                for j, c in enumerate(chunks):
                    s_matmul(s_t[:, j * QB : (j + 1) * QB], c, qsl)
                p_t = ppool.tile([128, GRP * QB], BF16, name="p_t")
                nc.scalar.activation(
                    p_t[:], s_t[:],
                    mybir.ActivationFunctionType.Exp, scale=SCALE,
                )
                pv_queue.append((qb, chunks, p_t))

            def finish_qb(qb):
                o_t = o_acc[qb]
                s_row = npool.tile([1, QB], BF16, name="s_row")
                nc.vector.tensor_copy(s_row[:], o_t[H : H + 1, :])
                # replicate sums across partitions on the PE (a rotating s_pool
                # slot; the DVE recip must read partitions 0:64 aligned)
                rep_ps = s_pool.tile([H, QB], F32, name="rep_ps", tag="s_t")
                nc.tensor.matmul(
                    rep_ps[:], ones_h[:], s_row[:], start=True, stop=True
                )
                r_rep = npool.tile([H, QB], F32, name="r_rep")
                nc.vector.reciprocal_approx_fast(out=r_rep[:], in_=rep_ps[:])
                o_n = npool.tile([H, QB], F32, name="o_n")
                nc.vector.tensor_mul(o_n[:], o_t[0:H, :], r_rep[:])
                nc.sync.dma_start(
                    out=outT[:, qb * QB : (qb + 1) * QB], in_=o_n[:]
                )

            def emit_available(nb, qbs):
                avail_pairs = 2 * (nb + 1)
                progress = True
                while progress:
                    progress = False
                    for qb in qbs:
                        if nb < qb:  # qb's queries come from x block qb
                            continue
                        if next_chunk[qb] < min(avail_pairs, NGROUPS):
                            emit_group(qb, next_chunk[qb])
                            next_chunk[qb] += 1
                            progress = True

            # --- production: x stream -> projections + qb0/1 attention ---
            x_tiles = {}

            def load_block(nb):
                if nb == 0:
                    x_tiles[0] = x_t0
                    return
                x_t = xpool.tile([128, ECH, QB], BF16, name="x_t")
                nc.sync.dma_start(
                    out=x_t[:], in_=xT[:, :, nb * QB : (nb + 1) * QB]
                )
                x_tiles[nb] = x_t

            load_block(0)
            load_block(1)
            load_block(2)
            for nb in range(NB):
                if nb + 3 < NB:
                    load_block(nb + 3)
                x_bf = x_tiles.pop(nb)
                want_q = nb < QBLKS
                # k/v matmuls split by chunk parity: even-chunk columns ->
                # partitions 0:64 (col group 0), odd-chunk columns -> 64:128,
                # so every chunk drains straight into its S row-group half.
                # x_bf viewed as [128, ec, 4 chunks, 128]; psum layout:
                #   [0:64,  0:256] = k chunks (even0, even1)
                #   [64:128,0:256] = k chunks (odd0, odd1)
                #   [64:128,256:512] = v chunks (even0, even1)
                #   [0:64,  256:512] = v chunks (odd0, odd1)
                x4 = x_bf.rearrange("p e (c t) -> p e c t", t=128)
                kv_ps = kvq_pool.tile([128, QB], F32, name="kv_ps", tag="kvq")
                # one weight chain per column-group at a time (interleaving k/v
                # chains within a group corrupts the accumulation)
                for ec in range(ECH):
                    first, last = ec == 0, ec == ECH - 1
                    nc.tensor.matmul(
                        kv_ps[0:H, 0:256], wT_sb[:, ec, 0:H],
                        x4[:, ec, 0:4:2, :],
                        start=first, stop=last, tile_position=(0, 0),
                    )
                    nc.tensor.matmul(
                        kv_ps[H:128, 0:256], wT_sb[:, ec, 0:H],
                        x4[:, ec, 1:4:2, :],
                        start=first, stop=last, tile_position=(0, H),
                    )
                for ec in range(ECH):
                    first, last = ec == 0, ec == ECH - 1
                    nc.tensor.matmul(
                        kv_ps[H:128, 256:512], wT_sb[:, ec, 2 * H : 3 * H],
                        x4[:, ec, 0:4:2, :],
                        start=first, stop=last, tile_position=(0, H),
                    )
                    nc.tensor.matmul(
                        kv_ps[0:H, 256:512], wT_sb[:, ec, 2 * H : 3 * H],
                        x4[:, ec, 1:4:2, :],
                        start=first, stop=last, tile_position=(0, 0),
                    )
                if nb == 0:
                    # more junk matmuls fill the kT-drain window so the HAM
                    # activity run started by warm(14) isn't broken
                    warm(8)
                # drains: strided copies scatter chunk columns back in place
                kT4 = kT_sb.rearrange("p (c t) -> p c t", t=128)
                cb = nb * 4
                nc.vector.tensor_copy(
                    kT4[0:H, cb : cb + 4 : 2, :],
                    kv_ps[0:H, 0:256].rearrange("p (c t) -> p c t", t=128),
                )
                nc.vector.tensor_copy(
                    kT4[H:128, cb + 1 : cb + 4 : 2, :],
                    kv_ps[H:128, 0:256].rearrange("p (c t) -> p c t", t=128),
                )
                if nb < 2:
                    # junk matmuls anchored on this block's kT drain (so Tile
                    # can't hoist them to t0): they keep the HAM activity run
                    # alive through the otherwise-cold early-stream region
                    wp = tr_pool.tile([128, 192], F32, name="warm_ps", tag="tr")
                    for _ in range(6):
                        nc.tensor.matmul(
                            wp[0:H, :], junk[:, 0:H],
                            kT_sb[:, nb * QB : nb * QB + 192],
                            start=True, stop=True, tile_position=(0, 0),
                        )
                if want_q:
                    # q computed into BOTH partition halves on the PE (same
                    # stationary weight through both column groups) so the S
                    # matmuls read a full query block from either half with no
                    # cross-duplication DMAs (SWDGE SBUF partition-shift
                    # measured ~7.5us to complete — far too slow)
                    q_ps = kvq_pool.tile([128, QB], F32, name="q_ps", tag="kvq")
                    # two SEQUENTIAL double-loops: a start=True matmul clears
                    # the has_written bits of its output PARTITIONS across the
                    # whole bank, so a partition range may host only ONE
                    # in-flight accumulation chain
                    for ec in range(ECH):
                        first, last = ec == 0, ec == ECH - 1
                        nc.tensor.matmul(
                            q_ps[0:H, 0:256], wT_sb[:, ec, H : 2 * H],
                            x4[:, ec, 0:4:2, :],
                            start=first, stop=last, tile_position=(0, 0),
                        )
                        nc.tensor.matmul(
                            q_ps[H:128, 256:512], wT_sb[:, ec, H : 2 * H],
                            x4[:, ec, 1:4:2, :],
                            start=first, stop=last, tile_position=(0, H),
                        )
                    for ec in range(ECH):
                        first, last = ec == 0, ec == ECH - 1
                        nc.tensor.matmul(
                            q_ps[H:128, 0:256], wT_sb[:, ec, H : 2 * H],
                            x4[:, ec, 0:4:2, :],
                            start=first, stop=last, tile_position=(0, H),
                        )
                        nc.tensor.matmul(
                            q_ps[0:H, 256:512], wT_sb[:, ec, H : 2 * H],
                            x4[:, ec, 1:4:2, :],
                            start=first, stop=last, tile_position=(0, 0),
                        )
                    qT4 = qT_sb.rearrange("p (c t) -> p c t", t=128)
                    qcb = nb * 4
                    nc.vector.tensor_copy(
                        qT4[:, qcb : qcb + 4 : 2, :],
                        q_ps[:, 0:256].rearrange("p (c t) -> p c t", t=128),
                    )
                    nc.vector.tensor_copy(
                        qT4[:, qcb + 1 : qcb + 4 : 2, :],
                        q_ps[:, 256:512].rearrange("p (c t) -> p c t", t=128),
                    )
                vT_blk = vstage.tile([128, QB], BF16)
                vT4 = vT_blk.rearrange("p (c t) -> p c t", t=128)
                nc.vector.tensor_copy(
                    vT4[H:128, 0:4:2, :],
                    kv_ps[H:128, 256:512].rearrange("p (c t) -> p c t", t=128),
                )
                nc.vector.tensor_copy(
                    vT4[0:H, 1:4:2, :],
                    kv_ps[0:H, 256:512].rearrange("p (c t) -> p c t", t=128),
                )
                # V-natural tiles via PE transpose (half follows chunk parity:
                # even chunks' v sits hi, odd chunks' v sits lo)
                for j in range(QB // 128):
                    c = nb * (QB // 128) + j
                    vlo = (j % 2) == 1
                    r = slice(0, H) if vlo else slice(H, 2 * H)
                    v_tr = tr_pool.tile([128, H], BF16, name="v_tr", tag="tr")
                    nc.tensor.transpose(
                        v_tr[:],
                        vT_blk[r, j * 128 : (j + 1) * 128],
                        ident[r, :],
                        tile_position=(0 if vlo else H, 0),
                    )
                    nc.vector.tensor_copy(v_all[:, c, 0:H], v_tr[:])
                emit_available(nb, (0, 1))
                if nb >= 4:
                    # late-stream blocks have PE slack (no q chain): drain
                    # some of the deferred PV backlog there
                    flush_pv(3)

            # --- drain: qb2/qb3 through the idle proj/transpose banks while
            # the deferred qb0/qb1 PV backlog interleaves into PE spare
            # cycles (2 flushes per group ~ balances PE vs ScalarE) ---
            for qb in (0, 1):
                while next_chunk[qb] < NGROUPS:
                    emit_group(qb, next_chunk[qb])
                    next_chunk[qb] += 1
                    flush_pv(1)
            o_acc[3] = kvq_pool.tile([128, QB], F32, name="o_qb3", tag="kvq")
            o_acc[2] = tr_pool.tile([128, QB], F32, name="o_qb2", tag="tr")
            # qb3 leads each round so its last exp lands ~one group earlier,
            # overlapping finish_qb(3)'s chain with qb2's final groups
            while next_chunk[2] < NGROUPS or next_chunk[3] < NGROUPS:
                for qb in (3, 2):
                    if next_chunk[qb] < NGROUPS:
                        emit_group(qb, next_chunk[qb])
                        next_chunk[qb] += 1
                        flush_pv(2)
            flush_pv_all()
            # qb0/qb1 finish early (their PVs completed mid-drain), hiding
            # under qb2/qb3's final exps
            finish_qb(0)
            finish_qb(1)
            finish_qb(3)
            finish_qb(2)

    nc.compile()
    return nc


_NC_CACHE = {}


def _get_nc():
    if "nc" not in _NC_CACHE:
        _NC_CACHE["nc"] = build_kernel()
    return _NC_CACHE["nc"]


def _make_in_maps(x, Wk, Wq, Wv):
    wT = np.ascontiguousarray(
        np.concatenate([Wk.T, Wq.T, Wv.T], axis=1)
    ).astype(ml_dtypes.bfloat16)
    in_maps = []
    for c in range(NCORES):
        b, h = divmod(c, 2)
        xb = np.asarray(x[b], dtype=np.float32)
        if h == 1:
            xb = np.concatenate([xb[NQ:], xb[:NQ]], axis=0)
        xbT = np.ascontiguousarray(xb.T).astype(ml_dtypes.bfloat16)
        in_maps.append({"xT": xbT, "wT": wT})
    return in_maps


def kernel(x, Wk, Wq, Wv, _trace=False, _tmpdir=None):
    nc = _get_nc()
    in_maps = _make_in_maps(x, Wk, Wq, Wv)
    kwargs = {}
    if _trace:
        kwargs = dict(trace=True, tmpdir=_tmpdir or tempfile.mkdtemp())
    res = run_bass_kernel_spmd(nc, in_maps, core_ids=list(range(NCORES)), **kwargs)
    out = np.empty((B, N, H), np.float32)
    for c in range(NCORES):
        b, h = divmod(c, 2)
        out[b, h * NQ : (h + 1) * NQ, :] = res.results[c]["outT"].T
    if _trace:
        return out, res
    return out


# revision 46
# speedup vs baseline: 1.0074x; 1.0074x over previous
"""Single-head attention (B=4, N=4096, E=1024, H=64) on 8 TRN2 NeuronCores.

Sharding: core c = (batch b = c//2, query-half h = c%2). Each core computes the
full K/V projections for its batch and attention for its 2048 query rows.
Attention is permutation-invariant over keys, so each core receives its batch's
x pre-transposed ([E, N], embedding on partitions) with its OWN query half in
columns 0:2048 - the program is identical across cores (pure SPMD), only the
data differs.

The kernel is ScalarE-bound: softmax needs exp over 8.4M elements per core and
exp only exists on ScalarE (1 elem/cycle/lane @1.2GHz) - a hard ~73us floor at
GRP=2 activation sizing. Everything else is organized to hide under that:
  - x and wT ship from the host already in bf16 (same rounding the device cast
    produced before): the x stream halves to 8 MiB (~26us) and the on-chip
    fp32->bf16 casts disappear entirely.
  - ScalarE does NOTHING but exp: the q cross-duplication DMAs are replaced by
    computing q into both PSUM partition halves on the PE (4 weight chains,
    same stationary weight twice per ec), and softmax denominators are
    replicated across partitions with a K=1 PE matmul into the (bank-sized) O
    accumulator's free partition half instead of a DRAM broadcast bounce.
  - per block, k/q/v projections are column-packed 2x on the PE: each matmul
    is split by 128-column sub-chunk parity, even sub-chunks through column
    group 0 (PSUM partitions 0:64), odd through group 1 (64:128), so each key
    chunk drains straight into the partition half its S-matmul row-group reads.
    vT is PE-transposed into V-natural [128, 65] tiles with a fused ones
    column (softmax denominators ride the PV accumulation).
  - S^T groups pair an even chunk (rows 0:64) with its odd neighbor (64:128):
    the two K=64 matmuls run concurrently in the 128x128 array. One
    exp((q.k)/8) pass per group on ScalarE -> bf16 P; PV accumulates
    O[65, 512] += [V|1].T @ P, lagging one group so the PE FIFO never blocks
    on ScalarE.
  - query blocks 0/1 ride chunk availability during the x stream (with the
    halved stream ScalarE saturates from ~6us on just these); qb2/qb3 follow
    through the same pools in the drain, qb3's O reusing the idle proj bank.
  - ~24 junk matmuls + one junk exp at t0 open the HAM clock gate and load the
    exp spline tables off the critical path.
Host assembles out[b, half] = outT.T.
"""

import tempfile

import ml_dtypes
import numpy as np

import concourse.bass as bass
import concourse.tile as tile
from concourse import bacc, mybir
from concourse.bass_utils import run_bass_kernel_spmd
from concourse.masks import make_identity

B, N, E, H = 4, 4096, 1024, 64
NCORES = 8
NQ = N // 2  # query rows per core
QB = 512  # query block (free dim of attention matmuls)
NKC = N // 128  # 32 key chunks of 128
ECH = E // 128  # 8 embedding chunks of 128
NB = N // QB  # 8 projection column blocks
QBLKS = NQ // QB  # 4 query blocks per core
GRP = 2  # key chunks per S/exp group (PSUM banks per S tile)
NGROUPS = NKC // GRP  # 16 S/exp groups per query block

F32 = mybir.dt.float32
BF16 = mybir.dt.bfloat16

SCALE = 1.0 / np.sqrt(H)


def build_kernel():
    nc = bacc.Bacc("TRN2", target_bir_lowering=False, debug=False, num_devices=NCORES)

    xT_d = nc.dram_tensor("xT", [E, N], BF16, kind="ExternalInput")
    wT_d = nc.dram_tensor("wT", [E, 3 * H], BF16, kind="ExternalInput")
    outT_d = nc.dram_tensor("outT", [H, NQ], F32, kind="ExternalOutput")

    xT = xT_d.ap().rearrange("(c p) n -> p c n", p=128)  # [128, ECH, N]
    wT = wT_d.ap().rearrange("(c p) h -> p c h", p=128)  # [128, ECH, 192]
    outT = outT_d.ap()

    with tile.TileContext(nc) as tc:
        with (
            tc.tile_pool(name="singles", bufs=1) as singles,
            tc.tile_pool(name="xpool", bufs=4) as xpool,
            tc.tile_pool(name="qkv", bufs=1) as qkv,
            tc.tile_pool(name="vstage", bufs=2) as vstage,
            tc.tile_pool(name="ppool", bufs=42) as ppool,
            tc.tile_pool(name="npool", bufs=2) as npool,
            tc.tile_pool(name="kvq_ps", bufs=1, space="PSUM") as kvq_pool,
            tc.tile_pool(name="tr_ps", bufs=1, space="PSUM") as tr_pool,
            tc.tile_pool(name="s_ps", bufs=2, space="PSUM") as s_pool,
            tc.tile_pool(name="o_ps", bufs=2, space="PSUM") as o_pool,
        ):
            # x block 0 streams in quarter-pieces with wT slotted after the
            # first quarter: the kv chain consumes (wT, quarter0) first, so
            # this order lets it start ~1us earlier than wT-first
            # wT rides the (otherwise idle pre-exp) scalar HWDGE queue so it
            # lands in parallel with x block 0's quarters on the sync queue —
            # the kv chain consumes both
            wT_sb = singles.tile([128, ECH, 3 * H], BF16)
            nc.scalar.dma_start(out=wT_sb[:], in_=wT)
            x_t0 = xpool.tile([128, ECH, QB], BF16, name="x_t")
            for piece in range(4):
                nc.sync.dma_start(
                    out=x_t0[:, 2 * piece : 2 * piece + 2, :],
                    in_=xT[:, 2 * piece : 2 * piece + 2, 0:QB],
                )
            # identity (both halves) for PE transposes of vT
            ident = singles.tile([128, H], BF16)
            make_identity(nc, ident[0:H, :])
            nc.scalar.dma_start(out=ident[H : 2 * H, :], in_=ident[0:H, :])

            # persistent activations: k/q in both partition halves by layout,
            # V-natural tiles with a fused ones column
            kT_sb = qkv.tile([128, N], BF16)
            qT_sb = qkv.tile([128, NQ], BF16)
            # chunk stride padded to 80 (transposing DMA needs aligned outputs)
            v_all = qkv.tile([128, NKC, 80], BF16)
            nc.vector.memset(v_all[:, :, H : H + 1], 1.0)

            ones_h = singles.tile([1, H], BF16)
            nc.vector.memset(ones_h[:], 1.0)

            # PE warmup from t~0: junk matmuls on a memset tile (not gated
            # by the wT DMA) so the HAM clock-gate opens before real work;
            # one junk exp loads the ACT spline tables off the critical path
            junk = singles.tile([128, 256], BF16)
            nc.vector.memset(junk[:], 0.5)
            warm_act = singles.tile([1, 128], BF16)
            nc.scalar.activation(
                warm_act[:], junk[0:1, 0:128],
                mybir.ActivationFunctionType.Exp, scale=SCALE,
            )
            # warm into the transpose bank: the kvq bank must stay free so the
            # first projection chain isn't queued behind junk. 14 matmuls
            # bridge the PE to the first DMA-paced kv chain; 8 more are
            # emitted after it (filling the kT-drain window) so the HAM
            # activity window sees ~4us of sustained work and opens the clock
            # gate before the q/S chains
            warm_ps = tr_pool.tile([128, 192], F32, name="warm_ps", tag="tr")

            def warm(n):
                for _ in range(n):
                    nc.tensor.matmul(
                        warm_ps[0:H, :], junk[:, 0:H], junk[:, 64:256],
                        start=True, stop=True, tile_position=(0, 0),
                    )

            warm(14)

            o_acc = [None] * QBLKS
            next_chunk = [0] * QBLKS
            # PV work is deferred through a FIFO: the stream phase emits only
            # S+exp (keeping ScalarE the pacer) and PVs flush into the PE's
            # spare cycles — opportunistically late in the stream, then two
            # per group in the drain. The newest entry is never flushed so
            # the PE FIFO can't block on the in-flight exp.
            pv_queue = []

            def flush_pv(n, qb_max=3):
                # qb_max<3 restricts stream-phase flushes to qb0/qb1, whose O
                # banks (o_pool) are free during the stream — qb2/qb3's O
                # lives in the kvq/tr banks, free only after the stream
                flushed = 0
                i = 0
                while flushed < n and i < len(pv_queue) - 1:
                    if pv_queue[i][0] <= qb_max:
                        emit_pv(*pv_queue.pop(i))
                        flushed += 1
                    else:
                        i += 1

            def flush_pv_all():
                while pv_queue:
                    emit_pv(*pv_queue.pop(0))

            def s_matmul(s_slice, c, qsl):
                # even chunks live in partitions 0:64, odd chunks in 64:128
                lo = c % 2 == 0
                r = slice(0, H) if lo else slice(H, 2 * H)
                nc.tensor.matmul(
                    s_slice,
                    kT_sb[r, c * 128 : (c + 1) * 128],
                    qT_sb[r, qsl],
                    start=True, stop=True,
                    tile_position=(0 if lo else H, 0),
                )

            def group_chunks(i):
                # adjacent chunks pair lo/hi row-groups within one x block
                return [2 * i, 2 * i + 1]

            def emit_pv(qb, chunks, p_t):
                for j, c in enumerate(chunks):
                    nc.tensor.matmul(
                        o_acc[qb][:],
                        v_all[:, c, 0 : H + 1],
                        p_t[:, j * QB : (j + 1) * QB],
                        start=(c == 0), stop=(c == NKC - 1),
                    )

            def emit_group(qb, i):
                if o_acc[qb] is None:
                    o_acc[qb] = o_pool.tile(
                        [H + 1, QB], F32, name=f"o_qb{qb}", tag="o_acc"
                    )
                qsl = slice(qb * QB, (qb + 1) * QB)
                chunks = group_chunks(i)
                s_t = s_pool.tile([128, GRP * QB], F32, name="s_t")
                for j, c in enumerate(chunks):
                    s_matmul(s_t[:, j * QB : (j + 1) * QB], c, qsl)
                p_t = ppool.tile([128, GRP * QB], BF16, name="p_t")
                nc.scalar.activation(
                    p_t[:], s_t[:],
                    mybir.ActivationFunctionType.Exp, scale=SCALE,
                )
                pv_queue.append((qb, chunks, p_t))

            def finish_qb(qb):
                o_t = o_acc[qb]
                s_row = npool.tile([1, QB], BF16, name="s_row")
                nc.vector.tensor_copy(s_row[:], o_t[H : H + 1, :])
                # replicate sums across partitions on the PE (a rotating s_pool
                # slot; the DVE recip must read partitions 0:64 aligned)
                rep_ps = s_pool.tile([H, QB], F32, name="rep_ps", tag="s_t")
                nc.tensor.matmul(
                    rep_ps[:], ones_h[:], s_row[:], start=True, stop=True
                )
                r_rep = npool.tile([H, QB], F32, name="r_rep")
                nc.vector.reciprocal_approx_fast(out=r_rep[:], in_=rep_ps[:])
                o_n = npool.tile([H, QB], F32, name="o_n")
                nc.vector.tensor_mul(o_n[:], o_t[0:H, :], r_rep[:])
                nc.sync.dma_start(
                    out=outT[:, qb * QB : (qb + 1) * QB], in_=o_n[:]
                )

            def emit_available(nb, qbs):
                avail_pairs = 2 * (nb + 1)
                progress = True
                while progress:
                    progress = False
                    for qb in qbs:
                        if nb < qb:  # qb's queries come from x block qb
                            continue
                        if next_chunk[qb] < min(avail_pairs, NGROUPS):
                            emit_group(qb, next_chunk[qb])
                            next_chunk[qb] += 1
                            progress = True

            # --- production: x stream -> projections + qb0/1 attention ---
            x_tiles = {}

            def load_block(nb):
                if nb == 0:
                    x_tiles[0] = x_t0
                    return
                x_t = xpool.tile([128, ECH, QB], BF16, name="x_t")
                nc.sync.dma_start(
                    out=x_t[:], in_=xT[:, :, nb * QB : (nb + 1) * QB]
                )
                x_tiles[nb] = x_t

            load_block(0)
            load_block(1)
            load_block(2)
            for nb in range(NB):
                if nb + 3 < NB:
                    load_block(nb + 3)
                x_bf = x_tiles.pop(nb)
                want_q = nb < QBLKS
                # k/v matmuls split by chunk parity: even-chunk columns ->
                # partitions 0:64 (col group 0), odd-chunk columns -> 64:128,
                # so every chunk drains straight into its S row-group half.
                # x_bf viewed as [128, ec, 4 chunks, 128]; psum layout:
                #   [0:64,  0:256] = k chunks (even0, even1)
                #   [64:128,0:256] = k chunks (odd0, odd1)
                #   [64:128,256:512] = v chunks (even0, even1)
                #   [0:64,  256:512] = v chunks (odd0, odd1)
                x4 = x_bf.rearrange("p e (c t) -> p e c t", t=128)
                kv_ps = kvq_pool.tile([128, QB], F32, name="kv_ps", tag="kvq")
                # one weight chain per column-group at a time (interleaving k/v
                # chains within a group corrupts the accumulation)
                for ec in range(ECH):
                    first, last = ec == 0, ec == ECH - 1
                    nc.tensor.matmul(
                        kv_ps[0:H, 0:256], wT_sb[:, ec, 0:H],
                        x4[:, ec, 0:4:2, :],
                        start=first, stop=last, tile_position=(0, 0),
                    )
                    nc.tensor.matmul(
                        kv_ps[H:128, 0:256], wT_sb[:, ec, 0:H],
                        x4[:, ec, 1:4:2, :],
                        start=first, stop=last, tile_position=(0, H),
                    )
                for ec in range(ECH):
                    first, last = ec == 0, ec == ECH - 1
                    nc.tensor.matmul(
                        kv_ps[H:128, 256:512], wT_sb[:, ec, 2 * H : 3 * H],
                        x4[:, ec, 0:4:2, :],
                        start=first, stop=last, tile_position=(0, H),
                    )
                    nc.tensor.matmul(
                        kv_ps[0:H, 256:512], wT_sb[:, ec, 2 * H : 3 * H],
                        x4[:, ec, 1:4:2, :],
                        start=first, stop=last, tile_position=(0, 0),
                    )
                if nb == 0:
                    # more junk matmuls fill the kT-drain window so the HAM
                    # activity run started by warm(14) isn't broken
                    warm(8)
                # drains: strided copies scatter chunk columns back in place
                kT4 = kT_sb.rearrange("p (c t) -> p c t", t=128)
                cb = nb * 4
                nc.vector.tensor_copy(
                    kT4[0:H, cb : cb + 4 : 2, :],
                    kv_ps[0:H, 0:256].rearrange("p (c t) -> p c t", t=128),
                )
                nc.vector.tensor_copy(
                    kT4[H:128, cb + 1 : cb + 4 : 2, :],
                    kv_ps[H:128, 0:256].rearrange("p (c t) -> p c t", t=128),
                )
                if nb < 2:
                    # junk matmuls anchored on this block's kT drain (so Tile
                    # can't hoist them to t0): they fill the PE's drain-wait
                    # window and keep the HAM activity run alive through the
                    # otherwise-cold early-stream region
                    wp = tr_pool.tile([128, 192], F32, name="warm_ps", tag="tr")
                    for _ in range(6):
                        nc.tensor.matmul(
                            wp[0:H, :], junk[:, 0:H],
                            kT_sb[:, nb * QB : nb * QB + 192],
                            start=True, stop=True, tile_position=(0, 0),
                        )
                if want_q:
                    # q computed into BOTH partition halves on the PE (same
                    # stationary weight through both column groups) so the S
                    # matmuls read a full query block from either half with no
                    # cross-duplication DMAs (SWDGE SBUF partition-shift
                    # measured ~7.5us to complete — far too slow)
                    q_ps = kvq_pool.tile([128, QB], F32, name="q_ps", tag="kvq")
                    # two SEQUENTIAL double-loops: a start=True matmul clears
                    # the has_written bits of its output PARTITIONS across the
                    # whole bank, so a partition range may host only ONE
                    # in-flight accumulation chain — each loop touches each
                    # partition half once; loop 2 starts after loop 1 stopped
                    for ec in range(ECH):
                        first, last = ec == 0, ec == ECH - 1
                        nc.tensor.matmul(
                            q_ps[0:H, 0:256], wT_sb[:, ec, H : 2 * H],
                            x4[:, ec, 0:4:2, :],
                            start=first, stop=last, tile_position=(0, 0),
                        )
                        nc.tensor.matmul(
                            q_ps[H:128, 256:512], wT_sb[:, ec, H : 2 * H],
                            x4[:, ec, 1:4:2, :],
                            start=first, stop=last, tile_position=(0, H),
                        )
                    for ec in range(ECH):
                        first, last = ec == 0, ec == ECH - 1
                        nc.tensor.matmul(
                            q_ps[H:128, 0:256], wT_sb[:, ec, H : 2 * H],
                            x4[:, ec, 0:4:2, :],
                            start=first, stop=last, tile_position=(0, H),
                        )
                        nc.tensor.matmul(
                            q_ps[0:H, 256:512], wT_sb[:, ec, H : 2 * H],
                            x4[:, ec, 1:4:2, :],
                            start=first, stop=last, tile_position=(0, 0),
                        )
                    qT4 = qT_sb.rearrange("p (c t) -> p c t", t=128)
                    qcb = nb * 4
                    nc.vector.tensor_copy(
                        qT4[:, qcb : qcb + 4 : 2, :],
                        q_ps[:, 0:256].rearrange("p (c t) -> p c t", t=128),
                    )
                    nc.vector.tensor_copy(
                        qT4[:, qcb + 1 : qcb + 4 : 2, :],
                        q_ps[:, 256:512].rearrange("p (c t) -> p c t", t=128),
                    )
                vT_blk = vstage.tile([128, QB], BF16)
                vT4 = vT_blk.rearrange("p (c t) -> p c t", t=128)
                nc.vector.tensor_copy(
                    vT4[H:128, 0:4:2, :],
                    kv_ps[H:128, 256:512].rearrange("p (c t) -> p c t", t=128),
                )
                nc.vector.tensor_copy(
                    vT4[0:H, 1:4:2, :],
                    kv_ps[0:H, 256:512].rearrange("p (c t) -> p c t", t=128),
                )
                # V-natural tiles via PE transpose (half follows chunk parity:
                # even chunks' v sits hi, odd chunks' v sits lo)
                for j in range(QB // 128):
                    c = nb * (QB // 128) + j
                    vlo = (j % 2) == 1
                    r = slice(0, H) if vlo else slice(H, 2 * H)
                    v_tr = tr_pool.tile([128, H], BF16, name="v_tr", tag="tr")
                    nc.tensor.transpose(
                        v_tr[:],
                        vT_blk[r, j * 128 : (j + 1) * 128],
                        ident[r, :],
                        tile_position=(0 if vlo else H, 0),
                    )
                    nc.vector.tensor_copy(v_all[:, c, 0:H], v_tr[:])
                # ALL four query blocks ride S+exp during the stream (their
                # PVs are deferred, so no O banks are needed yet) — this is
                # what keeps ScalarE fed past the per-block chunk-availability
                # limit of 2-rider scheduling
                emit_available(nb, (0, 1, 2, 3))
                if nb >= 2:
                    # blocks with PE slack drain the qb0/qb1 PV backlog
                    flush_pv(4, qb_max=1)

            # --- drain: the kvq/tr banks are free now, so qb2/qb3's deferred
            # PVs (and any leftover groups) interleave with the remaining
            # exp backlog at ~2 PV flushes per group ---
            o_acc[3] = kvq_pool.tile([H + 1, QB], F32, name="o_qb3", tag="kvq")
            o_acc[2] = tr_pool.tile([H + 1, QB], F32, name="o_qb2", tag="tr")
            for qb in (0, 1):
                while next_chunk[qb] < NGROUPS:
                    emit_group(qb, next_chunk[qb])
                    next_chunk[qb] += 1
                    flush_pv(2)
            while next_chunk[2] < NGROUPS or next_chunk[3] < NGROUPS:
                for qb in (3, 2):
                    if next_chunk[qb] < NGROUPS:
                        emit_group(qb, next_chunk[qb])
                        next_chunk[qb] += 1
                        flush_pv(2)
            flushed_mid = False
            while len(pv_queue) > 6:
                flush_pv(2)
                if not flushed_mid and not any(e[0] <= 1 for e in pv_queue):
                    # qb0/qb1's PVs have all flushed: their finish chains
                    # slot here, hiding under the PV/exp tail
                    flushed_mid = True
                    finish_qb(0)
                    finish_qb(1)
            flush_pv_all()
            if not flushed_mid:
                finish_qb(0)
                finish_qb(1)
            finish_qb(3)
            finish_qb(2)

    nc.compile()
    return nc


_NC_CACHE = {}


def _get_nc():
    if "nc" not in _NC_CACHE:
        _NC_CACHE["nc"] = build_kernel()
    return _NC_CACHE["nc"]


def _make_in_maps(x, Wk, Wq, Wv):
    wT = np.ascontiguousarray(
        np.concatenate([Wk.T, Wq.T, Wv.T], axis=1)
    ).astype(ml_dtypes.bfloat16)
    in_maps = []
    for c in range(NCORES):
        b, h = divmod(c, 2)
        xb = np.asarray(x[b], dtype=np.float32)
        if h == 1:
            xb = np.concatenate([xb[NQ:], xb[:NQ]], axis=0)
        xbT = np.ascontiguousarray(xb.T).astype(ml_dtypes.bfloat16)
        in_maps.append({"xT": xbT, "wT": wT})
    return in_maps


def kernel(x, Wk, Wq, Wv, _trace=False, _tmpdir=None):
    nc = _get_nc()
    in_maps = _make_in_maps(x, Wk, Wq, Wv)
    kwargs = {}
    if _trace:
        kwargs = dict(trace=True, tmpdir=_tmpdir or tempfile.mkdtemp())
    res = run_bass_kernel_spmd(nc, in_maps, core_ids=list(range(NCORES)), **kwargs)
    out = np.empty((B, N, H), np.float32)
    for c in range(NCORES):
        b, h = divmod(c, 2)
        out[b, h * NQ : (h + 1) * NQ, :] = res.results[c]["outT"].T
    if _trace:
        return out, res
    return out


# revision 48
# speedup vs baseline: 1.0269x; 1.0194x over previous
"""Single-head attention (B=4, N=4096, E=1024, H=64) on 8 TRN2 NeuronCores.

Sharding: core c = (batch b = c//2, query-half h = c%2). Each core computes the
full K/V projections for its batch and attention for its 2048 query rows.
Attention is permutation-invariant over keys, so each core receives its batch's
x pre-transposed ([E, N], embedding on partitions) with its OWN query half in
columns 0:2048 - the program is identical across cores (pure SPMD), only the
data differs.

The kernel is ScalarE-bound: softmax needs exp over 8.4M elements per core and
exp only exists on ScalarE (1 elem/cycle/lane @1.2GHz) - a hard ~73us floor at
GRP=2 activation sizing. Everything else is organized to hide under that:
  - x and wT ship from the host already in bf16 (same rounding the device cast
    produced before): the x stream halves to 8 MiB (~26us) and the on-chip
    fp32->bf16 casts disappear entirely.
  - ScalarE does NOTHING but exp: the q cross-duplication DMAs are replaced by
    computing q into both PSUM partition halves on the PE (4 weight chains,
    same stationary weight twice per ec), and softmax denominators are
    replicated across partitions with a K=1 PE matmul into the (bank-sized) O
    accumulator's free partition half instead of a DRAM broadcast bounce.
  - per block, k/q/v projections are column-packed 2x on the PE: each matmul
    is split by 128-column sub-chunk parity, even sub-chunks through column
    group 0 (PSUM partitions 0:64), odd through group 1 (64:128), so each key
    chunk drains straight into the partition half its S-matmul row-group reads.
    vT is PE-transposed into V-natural [128, 65] tiles with a fused ones
    column (softmax denominators ride the PV accumulation).
  - S^T groups pair an even chunk (rows 0:64) with its odd neighbor (64:128):
    the two K=64 matmuls run concurrently in the 128x128 array. One
    exp((q.k)/8) pass per group on ScalarE -> bf16 P; PV accumulates
    O[65, 512] += [V|1].T @ P, lagging one group so the PE FIFO never blocks
    on ScalarE.
  - query blocks 0/1 ride chunk availability during the x stream (with the
    halved stream ScalarE saturates from ~6us on just these); qb2/qb3 follow
    through the same pools in the drain, qb3's O reusing the idle proj bank.
  - ~24 junk matmuls + one junk exp at t0 open the HAM clock gate and load the
    exp spline tables off the critical path.
Host assembles out[b, half] = outT.T.
"""

import tempfile

import ml_dtypes
import numpy as np

import concourse.bass as bass
import concourse.tile as tile
from concourse import bacc, mybir
from concourse.bass_utils import run_bass_kernel_spmd
from concourse.masks import make_identity

B, N, E, H = 4, 4096, 1024, 64
NCORES = 8
NQ = N // 2  # query rows per core
QB = 512  # query block (free dim of attention matmuls)
NKC = N // 128  # 32 key chunks of 128
ECH = E // 128  # 8 embedding chunks of 128
NB = N // QB  # 8 projection column blocks
QBLKS = NQ // QB  # 4 query blocks per core
GRP = 2  # key chunks per S/exp group (PSUM banks per S tile)
NGROUPS = NKC // GRP  # 16 S/exp groups per query block

F32 = mybir.dt.float32
BF16 = mybir.dt.bfloat16

SCALE = 1.0 / np.sqrt(H)


def build_kernel():
    nc = bacc.Bacc("TRN2", target_bir_lowering=False, debug=False, num_devices=NCORES)

    xT_d = nc.dram_tensor("xT", [E, N], BF16, kind="ExternalInput")
    wT_d = nc.dram_tensor("wT", [E, 3 * H], BF16, kind="ExternalInput")
    outT_d = nc.dram_tensor("outT", [H, NQ], F32, kind="ExternalOutput")

    xT = xT_d.ap().rearrange("(c p) n -> p c n", p=128)  # [128, ECH, N]
    wT = wT_d.ap().rearrange("(c p) h -> p c h", p=128)  # [128, ECH, 192]
    outT = outT_d.ap()

    with tile.TileContext(nc) as tc:
        with (
            tc.tile_pool(name="singles", bufs=1) as singles,
            tc.tile_pool(name="xpool", bufs=4) as xpool,
            tc.tile_pool(name="qkv", bufs=1) as qkv,
            tc.tile_pool(name="vstage", bufs=2) as vstage,
            tc.tile_pool(name="ppool", bufs=42) as ppool,
            tc.tile_pool(name="npool", bufs=2) as npool,
            tc.tile_pool(name="kvq_ps", bufs=1, space="PSUM") as kvq_pool,
            tc.tile_pool(name="tr_ps", bufs=1, space="PSUM") as tr_pool,
            tc.tile_pool(name="s_ps", bufs=2, space="PSUM") as s_pool,
            tc.tile_pool(name="o_ps", bufs=2, space="PSUM") as o_pool,
        ):
            # x block 0 streams in quarter-pieces with wT slotted after the
            # first quarter: the kv chain consumes (wT, quarter0) first, so
            # this order lets it start ~1us earlier than wT-first
            # wT rides the (otherwise idle pre-exp) scalar HWDGE queue so it
            # lands in parallel with x block 0's quarters on the sync queue —
            # the kv chain consumes both
            wT_sb = singles.tile([128, ECH, 3 * H], BF16)
            nc.scalar.dma_start(out=wT_sb[:], in_=wT)
            x_t0 = xpool.tile([128, ECH, QB], BF16, name="x_t")
            for piece in range(4):
                nc.sync.dma_start(
                    out=x_t0[:, 2 * piece : 2 * piece + 2, :],
                    in_=xT[:, 2 * piece : 2 * piece + 2, 0:QB],
                )
            # identity (both halves) for PE transposes of vT
            ident = singles.tile([128, H], BF16)
            make_identity(nc, ident[0:H, :])
            nc.scalar.dma_start(out=ident[H : 2 * H, :], in_=ident[0:H, :])

            # persistent activations: k/q in both partition halves by layout,
            # V-natural tiles with a fused ones column
            kT_sb = qkv.tile([128, N], BF16)
            qT_sb = qkv.tile([128, NQ], BF16)
            # chunk stride padded to 128 with zeros past the ones column: the
            # PV stationary becomes a full 128-column weight, which satisfies
            # the compiler's FWL (fast weight load) condition — the junk rows
            # 65:128 of the O banks accumulate 0*P and are never read
            v_all = qkv.tile([128, NKC, 128], BF16)
            nc.vector.memset(v_all[:, :, H : H + 1], 1.0)
            nc.vector.memset(v_all[:, :, H + 1 :], 0.0)

            ones_h = singles.tile([1, H], BF16)
            nc.vector.memset(ones_h[:], 1.0)

            # PE warmup from t~0: junk matmuls on a memset tile (not gated
            # by the wT DMA) so the HAM clock-gate opens before real work;
            # one junk exp loads the ACT spline tables off the critical path
            junk = singles.tile([128, 256], BF16)
            nc.vector.memset(junk[:], 0.5)
            warm_act = singles.tile([1, 128], BF16)
            nc.scalar.activation(
                warm_act[:], junk[0:1, 0:128],
                mybir.ActivationFunctionType.Exp, scale=SCALE,
            )
            # warm into the transpose bank: the kvq bank must stay free so the
            # first projection chain isn't queued behind junk. 14 matmuls
            # bridge the PE to the first DMA-paced kv chain; 8 more are
            # emitted after it (filling the kT-drain window) so the HAM
            # activity window sees ~4us of sustained work and opens the clock
            # gate before the q/S chains
            warm_ps = tr_pool.tile([128, 192], F32, name="warm_ps", tag="tr")

            def warm(n):
                for _ in range(n):
                    nc.tensor.matmul(
                        warm_ps[0:H, :], junk[:, 0:H], junk[:, 64:256],
                        start=True, stop=True, tile_position=(0, 0),
                    )

            warm(14)

            o_acc = [None] * QBLKS
            next_chunk = [0] * QBLKS
            # PV work is deferred through a FIFO: the stream phase emits only
            # S+exp (keeping ScalarE the pacer) and PVs flush into the PE's
            # spare cycles — opportunistically late in the stream, then two
            # per group in the drain. The newest entry is never flushed so
            # the PE FIFO can't block on the in-flight exp.
            pv_queue = []

            def flush_pv(n, qb_max=3):
                # qb_max<3 restricts stream-phase flushes to qb0/qb1, whose O
                # banks (o_pool) are free during the stream — qb2/qb3's O
                # lives in the kvq/tr banks, free only after the stream
                flushed = 0
                i = 0
                while flushed < n and i < len(pv_queue) - 1:
                    if pv_queue[i][0] <= qb_max:
                        emit_pv(*pv_queue.pop(i))
                        flushed += 1
                    else:
                        i += 1

            def flush_pv_all():
                while pv_queue:
                    emit_pv(*pv_queue.pop(0))

            def s_matmul(s_slice, c, qsl):
                # even chunks live in partitions 0:64, odd chunks in 64:128
                lo = c % 2 == 0
                r = slice(0, H) if lo else slice(H, 2 * H)
                nc.tensor.matmul(
                    s_slice,
                    kT_sb[r, c * 128 : (c + 1) * 128],
                    qT_sb[r, qsl],
                    start=True, stop=True,
                    tile_position=(0 if lo else H, 0),
                )

            def group_chunks(i):
                # adjacent chunks pair lo/hi row-groups within one x block
                return [2 * i, 2 * i + 1]

            def emit_pv(qb, chunks, p_t):
                for j, c in enumerate(chunks):
                    nc.tensor.matmul(
                        o_acc[qb][:],
                        v_all[:, c, :],
                        p_t[:, j * QB : (j + 1) * QB],
                        start=(c == 0), stop=(c == NKC - 1),
                    )

            def emit_group(qb, i):
                if o_acc[qb] is None:
                    o_acc[qb] = o_pool.tile(
                        [128, QB], F32, name=f"o_qb{qb}", tag="o_acc"
                    )
                qsl = slice(qb * QB, (qb + 1) * QB)
                chunks = group_chunks(i)
                s_t = s_pool.tile([128, GRP * QB], F32, name="s_t")
                for j, c in enumerate(chunks):
                    s_matmul(s_t[:, j * QB : (j + 1) * QB], c, qsl)
                p_t = ppool.tile([128, GRP * QB], BF16, name="p_t")
                nc.scalar.activation(
                    p_t[:], s_t[:],
                    mybir.ActivationFunctionType.Exp, scale=SCALE,
                )
                pv_queue.append((qb, chunks, p_t))

            def finish_qb(qb):
                o_t = o_acc[qb]
                s_row = npool.tile([1, QB], BF16, name="s_row")
                nc.vector.tensor_copy(s_row[:], o_t[H : H + 1, :])
                # replicate sums across partitions on the PE (a rotating s_pool
                # slot; the DVE recip must read partitions 0:64 aligned)
                rep_ps = s_pool.tile([H, QB], F32, name="rep_ps", tag="s_t")
                nc.tensor.matmul(
                    rep_ps[:], ones_h[:], s_row[:], start=True, stop=True
                )
                r_rep = npool.tile([H, QB], F32, name="r_rep")
                nc.vector.reciprocal_approx_fast(out=r_rep[:], in_=rep_ps[:])
                o_n = npool.tile([H, QB], F32, name="o_n")
                nc.vector.tensor_mul(o_n[:], o_t[0:H, :], r_rep[:])
                nc.sync.dma_start(
                    out=outT[:, qb * QB : (qb + 1) * QB], in_=o_n[:]
                )

            def emit_available(nb, qbs):
                avail_pairs = 2 * (nb + 1)
                progress = True
                while progress:
                    progress = False
                    for qb in qbs:
                        if nb < qb:  # qb's queries come from x block qb
                            continue
                        if next_chunk[qb] < min(avail_pairs, NGROUPS):
                            emit_group(qb, next_chunk[qb])
                            next_chunk[qb] += 1
                            progress = True

            # --- production: x stream -> projections + qb0/1 attention ---
            x_tiles = {}

            def load_block(nb):
                if nb == 0:
                    x_tiles[0] = x_t0
                    return
                x_t = xpool.tile([128, ECH, QB], BF16, name="x_t")
                nc.sync.dma_start(
                    out=x_t[:], in_=xT[:, :, nb * QB : (nb + 1) * QB]
                )
                x_tiles[nb] = x_t

            load_block(0)
            load_block(1)
            load_block(2)
            for nb in range(NB):
                if nb + 3 < NB:
                    load_block(nb + 3)
                x_bf = x_tiles.pop(nb)
                want_q = nb < QBLKS
                # k/v matmuls split by chunk parity: even-chunk columns ->
                # partitions 0:64 (col group 0), odd-chunk columns -> 64:128,
                # so every chunk drains straight into its S row-group half.
                # x_bf viewed as [128, ec, 4 chunks, 128]; psum layout:
                #   [0:64,  0:256] = k chunks (even0, even1)
                #   [64:128,0:256] = k chunks (odd0, odd1)
                #   [64:128,256:512] = v chunks (even0, even1)
                #   [0:64,  256:512] = v chunks (odd0, odd1)
                x4 = x_bf.rearrange("p e (c t) -> p e c t", t=128)
                kv_ps = kvq_pool.tile([128, QB], F32, name="kv_ps", tag="kvq")
                # one weight chain per column-group at a time (interleaving k/v
                # chains within a group corrupts the accumulation)
                for ec in range(ECH):
                    first, last = ec == 0, ec == ECH - 1
                    nc.tensor.matmul(
                        kv_ps[0:H, 0:256], wT_sb[:, ec, 0:H],
                        x4[:, ec, 0:4:2, :],
                        start=first, stop=last, tile_position=(0, 0),
                    )
                    nc.tensor.matmul(
                        kv_ps[H:128, 0:256], wT_sb[:, ec, 0:H],
                        x4[:, ec, 1:4:2, :],
                        start=first, stop=last, tile_position=(0, H),
                    )
                for ec in range(ECH):
                    first, last = ec == 0, ec == ECH - 1
                    nc.tensor.matmul(
                        kv_ps[H:128, 256:512], wT_sb[:, ec, 2 * H : 3 * H],
                        x4[:, ec, 0:4:2, :],
                        start=first, stop=last, tile_position=(0, H),
                    )
                    nc.tensor.matmul(
                        kv_ps[0:H, 256:512], wT_sb[:, ec, 2 * H : 3 * H],
                        x4[:, ec, 1:4:2, :],
                        start=first, stop=last, tile_position=(0, 0),
                    )
                if nb == 0:
                    # more junk matmuls fill the kT-drain window so the HAM
                    # activity run started by warm(14) isn't broken
                    warm(8)
                # drains: strided copies scatter chunk columns back in place
                kT4 = kT_sb.rearrange("p (c t) -> p c t", t=128)
                cb = nb * 4
                nc.vector.tensor_copy(
                    kT4[0:H, cb : cb + 4 : 2, :],
                    kv_ps[0:H, 0:256].rearrange("p (c t) -> p c t", t=128),
                )
                nc.vector.tensor_copy(
                    kT4[H:128, cb + 1 : cb + 4 : 2, :],
                    kv_ps[H:128, 0:256].rearrange("p (c t) -> p c t", t=128),
                )
                if nb < 2:
                    # junk matmuls anchored on this block's kT drain (so Tile
                    # can't hoist them to t0): they fill the PE's drain-wait
                    # window and keep the HAM activity run alive through the
                    # otherwise-cold early-stream region
                    wp = tr_pool.tile([128, 192], F32, name="warm_ps", tag="tr")
                    for _ in range(6):
                        nc.tensor.matmul(
                            wp[0:H, :], junk[:, 0:H],
                            kT_sb[:, nb * QB : nb * QB + 192],
                            start=True, stop=True, tile_position=(0, 0),
                        )
                if want_q:
                    # q computed into BOTH partition halves on the PE (same
                    # stationary weight through both column groups) so the S
                    # matmuls read a full query block from either half with no
                    # cross-duplication DMAs (SWDGE SBUF partition-shift
                    # measured ~7.5us to complete — far too slow)
                    q_ps = kvq_pool.tile([128, QB], F32, name="q_ps", tag="kvq")
                    # two SEQUENTIAL double-loops: a start=True matmul clears
                    # the has_written bits of its output PARTITIONS across the
                    # whole bank, so a partition range may host only ONE
                    # in-flight accumulation chain — each loop touches each
                    # partition half once; loop 2 starts after loop 1 stopped
                    for ec in range(ECH):
                        first, last = ec == 0, ec == ECH - 1
                        nc.tensor.matmul(
                            q_ps[0:H, 0:256], wT_sb[:, ec, H : 2 * H],
                            x4[:, ec, 0:4:2, :],
                            start=first, stop=last, tile_position=(0, 0),
                        )
                        nc.tensor.matmul(
                            q_ps[H:128, 256:512], wT_sb[:, ec, H : 2 * H],
                            x4[:, ec, 1:4:2, :],
                            start=first, stop=last, tile_position=(0, H),
                        )
                    for ec in range(ECH):
                        first, last = ec == 0, ec == ECH - 1
                        nc.tensor.matmul(
                            q_ps[H:128, 0:256], wT_sb[:, ec, H : 2 * H],
                            x4[:, ec, 0:4:2, :],
                            start=first, stop=last, tile_position=(0, H),
                        )
                        nc.tensor.matmul(
                            q_ps[0:H, 256:512], wT_sb[:, ec, H : 2 * H],
                            x4[:, ec, 1:4:2, :],
                            start=first, stop=last, tile_position=(0, 0),
                        )
                    qT4 = qT_sb.rearrange("p (c t) -> p c t", t=128)
                    qcb = nb * 4
                    nc.vector.tensor_copy(
                        qT4[:, qcb : qcb + 4 : 2, :],
                        q_ps[:, 0:256].rearrange("p (c t) -> p c t", t=128),
                    )
                    nc.vector.tensor_copy(
                        qT4[:, qcb + 1 : qcb + 4 : 2, :],
                        q_ps[:, 256:512].rearrange("p (c t) -> p c t", t=128),
                    )
                vT_blk = vstage.tile([128, QB], BF16)
                vT4 = vT_blk.rearrange("p (c t) -> p c t", t=128)
                nc.vector.tensor_copy(
                    vT4[H:128, 0:4:2, :],
                    kv_ps[H:128, 256:512].rearrange("p (c t) -> p c t", t=128),
                )
                nc.vector.tensor_copy(
                    vT4[0:H, 1:4:2, :],
                    kv_ps[0:H, 256:512].rearrange("p (c t) -> p c t", t=128),
                )
                # V-natural tiles via PE transpose (half follows chunk parity:
                # even chunks' v sits hi, odd chunks' v sits lo)
                for j in range(QB // 128):
                    c = nb * (QB // 128) + j
                    vlo = (j % 2) == 1
                    r = slice(0, H) if vlo else slice(H, 2 * H)
                    v_tr = tr_pool.tile([128, H], BF16, name="v_tr", tag="tr")
                    nc.tensor.transpose(
                        v_tr[:],
                        vT_blk[r, j * 128 : (j + 1) * 128],
                        ident[r, :],
                        tile_position=(0 if vlo else H, 0),
                    )
                    nc.vector.tensor_copy(v_all[:, c, 0:H], v_tr[:])
                # ALL four query blocks ride S+exp during the stream (their
                # PVs are deferred, so no O banks are needed yet) — this is
                # what keeps ScalarE fed past the per-block chunk-availability
                # limit of 2-rider scheduling
                emit_available(nb, (0, 1, 2, 3))
                if nb >= 2:
                    # blocks with PE slack drain the qb0/qb1 PV backlog
                    flush_pv(4, qb_max=1)

            # --- drain: the kvq/tr banks are free now, so qb2/qb3's deferred
            # PVs (and any leftover groups) interleave with the remaining
            # exp backlog at ~2 PV flushes per group ---
            o_acc[3] = kvq_pool.tile([128, QB], F32, name="o_qb3", tag="kvq")
            o_acc[2] = tr_pool.tile([128, QB], F32, name="o_qb2", tag="tr")
            for qb in (0, 1):
                while next_chunk[qb] < NGROUPS:
                    emit_group(qb, next_chunk[qb])
                    next_chunk[qb] += 1
                    flush_pv(2)
            while next_chunk[2] < NGROUPS or next_chunk[3] < NGROUPS:
                for qb in (3, 2):
                    if next_chunk[qb] < NGROUPS:
                        emit_group(qb, next_chunk[qb])
                        next_chunk[qb] += 1
                        flush_pv(2)
            flushed_mid = False
            while len(pv_queue) > 6:
                flush_pv(2)
                if not flushed_mid and not any(e[0] <= 1 for e in pv_queue):
                    # qb0/qb1's PVs have all flushed: their finish chains
                    # slot here, hiding under the PV/exp tail
                    flushed_mid = True
                    finish_qb(0)
                    finish_qb(1)
            flush_pv_all()
            if not flushed_mid:
                finish_qb(0)
                finish_qb(1)
            finish_qb(3)
            finish_qb(2)

    nc.compile()
    return nc


_NC_CACHE = {}


def _get_nc():
    if "nc" not in _NC_CACHE:
        _NC_CACHE["nc"] = build_kernel()
    return _NC_CACHE["nc"]


def _make_in_maps(x, Wk, Wq, Wv):
    wT = np.ascontiguousarray(
        np.concatenate([Wk.T, Wq.T, Wv.T], axis=1)
    ).astype(ml_dtypes.bfloat16)
    in_maps = []
    for c in range(NCORES):
        b, h = divmod(c, 2)
        xb = np.asarray(x[b], dtype=np.float32)
        if h == 1:
            xb = np.concatenate([xb[NQ:], xb[:NQ]], axis=0)
        xbT = np.ascontiguousarray(xb.T).astype(ml_dtypes.bfloat16)
        in_maps.append({"xT": xbT, "wT": wT})
    return in_maps


def kernel(x, Wk, Wq, Wv, _trace=False, _tmpdir=None):
    nc = _get_nc()
    in_maps = _make_in_maps(x, Wk, Wq, Wv)
    kwargs = {}
    if _trace:
        kwargs = dict(trace=True, tmpdir=_tmpdir or tempfile.mkdtemp())
    res = run_bass_kernel_spmd(nc, in_maps, core_ids=list(range(NCORES)), **kwargs)
    out = np.empty((B, N, H), np.float32)
    for c in range(NCORES):
        b, h = divmod(c, 2)
        out[b, h * NQ : (h + 1) * NQ, :] = res.results[c]["outT"].T
    if _trace:
        return out, res
    return out
